# revision 7
# baseline (speedup 1.0000x reference)
"""CapsuleLayer (dynamic routing) Trainium2 kernel, 8-core SPMD.

Sharding: n_in (2048) split 8 ways -> 256 capsules/core, full batch per core.
Kernel shard resident in SBUF (bf16). Routing sums S all-reduced across cores.
"""
import numpy as np
import ml_dtypes

import concourse.bacc as bacc
import concourse.tile as tile
import concourse.mybir as mybir
from concourse import bass_utils

N_CORES = 8
B = 64          # batch
NI = 2048       # n_in total
NS = NI // N_CORES  # 256 per core
K = 16          # d_in
J = 32          # n_out
D = 32          # d_out
JD = J * D      # 1024
NK = NS * K     # 4096 rows per core
NCH = NK // 128  # 32 chunks
NPAIR = NS // 2  # 128 pairs
EPS = 1e-8
ITERS = 3

F32 = mybir.dt.float32
BF16 = mybir.dt.bfloat16
AX = mybir.AxisListType
OP = mybir.AluOpType
ACTF = mybir.ActivationFunctionType

_CACHE = {}


def _build():
    if "nc" in _CACHE:
        return _CACHE["nc"]
    nc = bacc.Bacc("TRN2", target_bir_lowering=False, debug=False,
                   num_devices=N_CORES)
    kfh_in = nc.dram_tensor("kfh_in", [128, NCH * JD], BF16, kind="ExternalInput").ap()
    xt2_in = nc.dram_tensor("xt2_in", [128, 32 * 128], BF16, kind="ExternalInput").ap()
    xtd_in = nc.dram_tensor("xtd_in", [128, NCH * B], BF16, kind="ExternalInput").ap()
    id32_in = nc.dram_tensor("id32_in", [32, 32], F32, kind="ExternalInput").ap()
    v_out = nc.dram_tensor("v_out", [B, JD], F32, kind="ExternalOutput").ap()

    with tile.TileContext(nc) as tc:
        with (
            tc.tile_pool(name="sbuf", bufs=1) as sb,
            tc.tile_pool(name="stage", bufs=2) as st,
            tc.tile_pool(name="psum", bufs=2, space="PSUM") as ps,
            tc.tile_pool(name="psx", bufs=2, space="PSUM") as psx,
            tc.tile_pool(name="dram", bufs=1, space="DRAM") as dram,
        ):
            # ---- residents ----
            KF = sb.tile([128, NCH, JD], BF16)      # kernel shard [(nk)p, chunk, (jd)]
            XT2 = sb.tile([128, 32, 128], BF16)     # block-diag x pairs
            XTD = sb.tile([128, NCH, B], BF16)      # dense xT chunks
            ID32 = sb.tile([32, 32], F32)
            nc.sync.dma_start(KF[:], kfh_in.rearrange("p (c f) -> p c f", c=NCH))
            nc.sync.dma_start(XT2[:], xt2_in.rearrange("p (c f) -> p c f", c=32))
            nc.sync.dma_start(XTD[:], xtd_in.rearrange("p (c f) -> p c f", c=NCH))
            nc.sync.dma_start(ID32[:], id32_in)

            Sg = sb.tile([B, JD], F32)        # gathered routing sum
            Vt = sb.tile([B, JD], F32)        # current V
            Tt = sb.tile([B, JD], F32)        # running sum of V's
            Texp2 = sb.tile([128, JD], F32)   # T duplicated on both halves
            Dbuf = sb.tile([128, J, NPAIR], F32)
            Ebuf = sb.tile([128, J, NPAIR], BF16)
            smr = sb.tile([128, NPAIR], F32)
            wbuf = sb.tile([128, J, NPAIR], BF16)
            Sres = sb.tile([B, J, D], F32)

            def allreduce(src):
                ib = dram.tile([B, JD], F32, tag=f"arin{allreduce.n}")
                ob = dram.tile([B, JD], F32, tag=f"arout{allreduce.n}",
                               addr_space="Shared")
                allreduce.n += 1
                nc.sync.dma_start(ib[:], src)
                nc.gpsimd.collective_compute(
                    "AllReduce", OP.add, ins=[ib.opt()], outs=[ob.opt()],
                    replica_groups=[list(range(N_CORES))])
                nc.sync.dma_start(Sg[:], ob[:])

            allreduce.n = 0

            def squash():
                """Vt = squash(Sg) along d (inner 32)."""
                sq = st.tile([B, JD], F32, tag="sq")
                nc.vector.tensor_mul(sq[:], Sg[:], Sg[:])
                nrm = st.tile([B, J], F32, tag="nrm")
                nc.vector.tensor_reduce(nrm[:], sq[:].rearrange("b (j d) -> b j d", j=J),
                                        axis=AX.X, op=OP.add)
                u = st.tile([B, J], F32, tag="u")
                nc.vector.tensor_scalar_add(u[:], nrm[:], EPS)
                s0 = st.tile([B, J], F32, tag="s0")
                nc.scalar.activation(s0[:], u[:], ACTF.Sqrt)
                y0 = st.tile([B, J], F32, tag="y0")
                nc.vector.reciprocal(y0[:], s0[:])
                # newton refine rsqrt: y1 = y0*(1.5 - 0.5*u*y0^2)
                t1 = st.tile([B, J], F32, tag="t1")
                nc.vector.tensor_mul(t1[:], y0[:], y0[:])
                nc.vector.tensor_mul(t1[:], t1[:], u[:])
                nc.vector.tensor_scalar_mul(t1[:], t1[:], -0.5)
                nc.vector.tensor_scalar_add(t1[:], t1[:], 1.5)
                y1 = st.tile([B, J], F32, tag="y1")
                nc.vector.tensor_mul(y1[:], y0[:], t1[:])
                # g = nrm/(1+nrm) * y1
                t2 = st.tile([B, J], F32, tag="t2")
                nc.vector.tensor_scalar_add(t2[:], nrm[:], 1.0)
                r2 = st.tile([B, J], F32, tag="r2")
                nc.vector.reciprocal(r2[:], t2[:])
                g = st.tile([B, J], F32, tag="g")
                nc.vector.tensor_mul(g[:], nrm[:], r2[:])
                nc.vector.tensor_mul(g[:], g[:], y1[:])
                gb = g[:].unsqueeze(2).broadcast_to((B, J, D))
                nc.vector.tensor_mul(Vt[:].rearrange("b (j d) -> b j d", j=J),
                                     Sg[:].rearrange("b (j d) -> b j d", j=J), gb)

            # ---- iter 1: S1 = sum_n U / 32 ----
            s1p = ps.tile([128, JD], F32, tag="upsum")
            for c in range(NCH):
                for o in (0, 512):
                    nc.tensor.matmul(s1p[0:B, o:o + 512], XTD[:, c, :],
                                     KF[:, c, o:o + 512],
                                     start=(c == 0),
                                     stop=(c == NCH - 1))
            S1s = sb.tile([B, JD], F32)
            nc.vector.tensor_scalar_mul(S1s[:], s1p[0:B, :], 1.0 / J)
            allreduce(S1s[:])
            squash()
            nc.vector.tensor_copy(Tt[:], Vt[:])

            for it in range(ITERS - 1):
                # refresh Texp2 (T on both partition halves)
                nc.sync.dma_start(Texp2[0:B, :], Tt[:])
                nc.sync.dma_start(Texp2[B:128, :], Tt[:])

                # ---- pass A: D[b,n,j] = <U_n, T> per n (pairs of n) ----
                for p in range(NPAIR):
                    bp = (p % 4) * 32
                    g = p // 4
                    up = ps.tile([128, JD], F32, tag="upsum")
                    for o in (0, 512):
                        nc.tensor.matmul(up[:, o:o + 512], XT2[bp:bp + 32, g, :],
                                         KF[bp:bp + 32, g, o:o + 512],
                                         start=True, stop=True,
                                         tile_position=(bp, 0))
                    pr = st.tile([128, JD], BF16, tag="pstage")
                    nc.vector.tensor_mul(pr[:], up[:], Texp2[:])
                    nc.vector.tensor_reduce(
                        Dbuf[:, :, p],
                        pr[:].rearrange("p (j d) -> p j d", j=J),
                        axis=AX.X, op=OP.add)

                # ---- softmax over j -> weights C (no max-sub; logits small) ----
                nc.scalar.activation(Ebuf[:], Dbuf[:], ACTF.Exp)
                nc.vector.tensor_reduce(smr[:], Ebuf[:].transpose([0, 2, 1]),
                                        axis=AX.X, op=OP.add)
                rs = st.tile([128, NPAIR], F32, tag="rs")
                nc.vector.reciprocal(rs[:], smr[:])
                nc.vector.tensor_mul(wbuf[:], Ebuf[:],
                                     rs[:].unsqueeze(1).broadcast_to((128, J, NPAIR)))

                # ---- pass B: S = sum_n C*U via XC-folded matmuls per j ----
                for j in range(J):
                    wT = st.tile([128, NPAIR], BF16, tag="wT")
                    nc.sync.dma_start_transpose(wT[:], wbuf[:, j, :])
                    wTd = dram.tile([128, NPAIR], BF16, tag="wTd")
                    nc.sync.dma_start(wTd[:], wT[:])
                    wexp = st.tile([128, NCH, B], BF16, tag="wexp")
                    for nl in range(8):
                        pl, par = nl // 2, nl % 2
                        src = (wTd[pl:128:4, par * B:(par + 1) * B]
                               .unsqueeze(0).broadcast_to((K, NCH, B)))
                        nc.sync.dma_start(wexp[nl * K:(nl + 1) * K, :, :], src)
                    xcw = st.tile([128, NCH, B], BF16, tag="xcw")
                    nc.vector.tensor_mul(xcw[:], XTD[:], wexp[:])
                    sj = psx.tile([D, B], F32, tag="xc")
                    for c in range(NCH):
                        nc.tensor.matmul(sj[:], KF[:, c, j * D:(j + 1) * D],
                                         xcw[:, c, :],
                                         start=(c == 0), stop=(c == NCH - 1))
                    sjs = st.tile([D, B], F32, tag="sjs")
                    nc.vector.tensor_copy(sjs[:], sj[:])
                    stp = psx.tile([B, D], F32, tag="xct")
                    nc.tensor.transpose(stp[:], sjs[:], ID32[:])
                    nc.vector.tensor_copy(Sres[:, j, :], stp[:])

                allreduce(Sres[:].rearrange("b j d -> b (j d)"))
                squash()
                if it == 0:
                    nc.vector.tensor_add(Tt[:], Tt[:], Vt[:])

            nc.sync.dma_start(v_out, Vt[:])

    nc.compile()
    _CACHE["nc"] = nc
    return nc


def _prep_core(x, kern, c):
    """Host-side shard prep for core c. x [B,NI,K] f32, kern [NI,J,K,D] f32."""
    n0 = c * NS
    xs = x[:, n0:n0 + NS, :]                       # [B, NS, K]
    ks = kern[n0:n0 + NS]                          # [NS, J, K, D]
    # KFH [(n k), (j d)] -> [128, NCH*JD] chunk-major partitions
    kf = ks.transpose(0, 2, 1, 3).reshape(NK, JD)  # [(n k), (j d)]
    kf = kf.reshape(NCH, 128, JD).transpose(1, 0, 2).reshape(128, NCH * JD)
    # xTd [(n k), b] -> [128, NCH*B]
    xt = xs.transpose(1, 2, 0).reshape(NK, B)      # [(n k), b]
    xtd = xt.reshape(NCH, 128, B).transpose(1, 0, 2).reshape(128, NCH * B)
    # xT2 block-diag per pair: [32, 128] blocks; tile [128, 32, 128]
    xt2 = np.zeros((128, 32, 128), np.float32)
    for p in range(NPAIR):
        bp = (p % 4) * 32
        g = p // 4
        blk = np.zeros((32, 128), np.float32)
        for nl in range(2):
            n = 2 * p + nl
            # rows nl*16..+16 (k), cols nl*64..+64 (b)
            blk[nl * 16:(nl + 1) * 16, nl * B:(nl + 1) * B] = xs[:, n, :].T
        xt2[bp:bp + 32, g, :] = blk
    bf = ml_dtypes.bfloat16
    return {
        "kfh_in": kf.astype(bf),
        "xt2_in": xt2.reshape(128, 32 * 128).astype(bf),
        "xtd_in": xtd.astype(bf),
        "id32_in": np.eye(32, dtype=np.float32),
    }


def kernel(x, kernel):
    import os
    nc = _build()
    kern = kernel
    in_maps = [_prep_core(np.asarray(x, np.float32),
                          np.asarray(kern, np.float32), c)
               for c in range(N_CORES)]
    trace = bool(int(os.environ.get("KERNEL_TRACE", "0")))
    if trace:
        try:
            res = bass_utils.run_bass_kernel_spmd(
                nc, in_maps, core_ids=list(range(N_CORES)), trace=True)
            if res.exec_time_ns is not None:
                print(f"HW exec time: {res.exec_time_ns} ns")
        except Exception:
            trace = False
    if not trace:
        res = bass_utils.run_bass_kernel_spmd(
            nc, in_maps, core_ids=list(range(N_CORES)))
    out = res.results[0]["v_out"]
    return out.reshape(B, J, D).astype(np.float32)


# revision 8
# speedup vs baseline: 1.0022x; 1.0022x over previous
"""CapsuleLayer (dynamic routing) Trainium2 kernel, 8-core SPMD.

Sharding: n_in (2048) split 8 ways -> 256 capsules/core, full batch per core.
Kernel shard resident in SBUF (bf16). Routing sums S all-reduced across cores.
"""
import numpy as np
import ml_dtypes

import concourse.bacc as bacc
import concourse.tile as tile
import concourse.mybir as mybir
from concourse import bass_utils

N_CORES = 8
B = 64          # batch
NI = 2048       # n_in total
NS = NI // N_CORES  # 256 per core
K = 16          # d_in
J = 32          # n_out
D = 32          # d_out
JD = J * D      # 1024
NK = NS * K     # 4096 rows per core
NCH = NK // 128  # 32 chunks
NPAIR = NS // 2  # 128 pairs
EPS = 1e-8
ITERS = 3

F32 = mybir.dt.float32
BF16 = mybir.dt.bfloat16
AX = mybir.AxisListType
OP = mybir.AluOpType
ACTF = mybir.ActivationFunctionType

_CACHE = {}


def _build():
    if "nc" in _CACHE:
        return _CACHE["nc"]
    nc = bacc.Bacc("TRN2", target_bir_lowering=False, debug=False,
                   num_devices=N_CORES)
    kfh_in = nc.dram_tensor("kfh_in", [128, NCH * JD], BF16, kind="ExternalInput").ap()
    xt2_in = nc.dram_tensor("xt2_in", [128, 32 * 128], BF16, kind="ExternalInput").ap()
    xtd_in = nc.dram_tensor("xtd_in", [128, NCH * B], BF16, kind="ExternalInput").ap()
    id32_in = nc.dram_tensor("id32_in", [32, 32], F32, kind="ExternalInput").ap()
    v_out = nc.dram_tensor("v_out", [B, JD], F32, kind="ExternalOutput").ap()

    with tile.TileContext(nc) as tc:
        with (
            tc.tile_pool(name="sbuf", bufs=1) as sb,
            tc.tile_pool(name="stage", bufs=3) as st,
            tc.tile_pool(name="psum", bufs=2, space="PSUM") as ps,
            tc.tile_pool(name="psx", bufs=2, space="PSUM") as psx,
            tc.tile_pool(name="dram", bufs=2, space="DRAM") as dram,
        ):
            # ---- residents ----
            KF = sb.tile([128, NCH, JD], BF16)      # kernel shard [(nk)p, chunk, (jd)]
            XT2 = sb.tile([128, 32, 128], BF16)     # block-diag x pairs
            XTD = sb.tile([128, NCH, B], BF16)      # dense xT chunks
            ID32 = sb.tile([32, 32], F32)
            nc.sync.dma_start(KF[:], kfh_in.rearrange("p (c f) -> p c f", c=NCH))
            nc.sync.dma_start(XT2[:], xt2_in.rearrange("p (c f) -> p c f", c=32))
            nc.sync.dma_start(XTD[:], xtd_in.rearrange("p (c f) -> p c f", c=NCH))
            nc.sync.dma_start(ID32[:], id32_in)

            Sg = sb.tile([B, JD], F32)        # gathered routing sum
            Vt = sb.tile([B, JD], F32)        # current V
            Tt = sb.tile([B, JD], F32)        # running sum of V's
            Texp2 = sb.tile([128, JD], F32)   # T duplicated on both halves
            Dbuf = sb.tile([128, J, NPAIR], F32)
            Ebuf = sb.tile([128, J, NPAIR], BF16)
            smr = sb.tile([128, NPAIR], F32)
            wbuf = sb.tile([128, J, NPAIR], BF16)
            Sres = sb.tile([B, J, D], F32)

            def allreduce(src):
                ib = dram.tile([B, JD], F32, tag=f"arin{allreduce.n}")
                ob = dram.tile([B, JD], F32, tag=f"arout{allreduce.n}",
                               addr_space="Shared")
                allreduce.n += 1
                nc.sync.dma_start(ib[:], src)
                nc.gpsimd.collective_compute(
                    "AllReduce", OP.add, ins=[ib.opt()], outs=[ob.opt()],
                    replica_groups=[list(range(N_CORES))])
                nc.sync.dma_start(Sg[:], ob[:])

            allreduce.n = 0

            def squash():
                """Vt = squash(Sg) along d (inner 32)."""
                sq = st.tile([B, JD], F32, tag="sq")
                nc.vector.tensor_mul(sq[:], Sg[:], Sg[:])
                nrm = st.tile([B, J], F32, tag="nrm")
                nc.vector.tensor_reduce(nrm[:], sq[:].rearrange("b (j d) -> b j d", j=J),
                                        axis=AX.X, op=OP.add)
                u = st.tile([B, J], F32, tag="u")
                nc.vector.tensor_scalar_add(u[:], nrm[:], EPS)
                s0 = st.tile([B, J], F32, tag="s0")
                nc.scalar.activation(s0[:], u[:], ACTF.Sqrt)
                y0 = st.tile([B, J], F32, tag="y0")
                nc.vector.reciprocal(y0[:], s0[:])
                # newton refine rsqrt: y1 = y0*(1.5 - 0.5*u*y0^2)
                t1 = st.tile([B, J], F32, tag="t1")
                nc.vector.tensor_mul(t1[:], y0[:], y0[:])
                nc.vector.tensor_mul(t1[:], t1[:], u[:])
                nc.vector.tensor_scalar_mul(t1[:], t1[:], -0.5)
                nc.vector.tensor_scalar_add(t1[:], t1[:], 1.5)
                y1 = st.tile([B, J], F32, tag="y1")
                nc.vector.tensor_mul(y1[:], y0[:], t1[:])
                # g = nrm/(1+nrm) * y1
                t2 = st.tile([B, J], F32, tag="t2")
                nc.vector.tensor_scalar_add(t2[:], nrm[:], 1.0)
                r2 = st.tile([B, J], F32, tag="r2")
                nc.vector.reciprocal(r2[:], t2[:])
                g = st.tile([B, J], F32, tag="g")
                nc.vector.tensor_mul(g[:], nrm[:], r2[:])
                nc.vector.tensor_mul(g[:], g[:], y1[:])
                gb = g[:].unsqueeze(2).broadcast_to((B, J, D))
                nc.vector.tensor_mul(Vt[:].rearrange("b (j d) -> b j d", j=J),
                                     Sg[:].rearrange("b (j d) -> b j d", j=J), gb)

            # ---- iter 1: S1 = sum_n U / 32 ----
            s1p = ps.tile([128, JD], F32, tag="upsum")
            for c in range(NCH):
                for o in (0, 512):
                    nc.tensor.matmul(s1p[0:B, o:o + 512], XTD[:, c, :],
                                     KF[:, c, o:o + 512],
                                     start=(c == 0),
                                     stop=(c == NCH - 1))
            S1s = sb.tile([B, JD], F32)
            nc.vector.tensor_scalar_mul(S1s[:], s1p[0:B, :], 1.0 / J)
            allreduce(S1s[:])
            squash()
            nc.vector.tensor_copy(Tt[:], Vt[:])

            for it in range(ITERS - 1):
                # refresh Texp2 (T on both partition halves)
                nc.sync.dma_start(Texp2[0:B, :], Tt[:])
                nc.sync.dma_start(Texp2[B:128, :], Tt[:])

                # ---- pass A: D[b,n,j] = <U_n, T> per n (pairs of n) ----
                for p in range(NPAIR):
                    bp = (p % 4) * 32
                    g = p // 4
                    up = ps.tile([128, JD], F32, tag="upsum")
                    for o in (0, 512):
                        nc.tensor.matmul(up[:, o:o + 512], XT2[bp:bp + 32, g, :],
                                         KF[bp:bp + 32, g, o:o + 512],
                                         start=True, stop=True,
                                         tile_position=(bp, 0))
                    pr = st.tile([128, JD], BF16, tag="pstage")
                    nc.vector.tensor_mul(pr[:], up[:], Texp2[:])
                    nc.vector.tensor_reduce(
                        Dbuf[:, :, p],
                        pr[:].rearrange("p (j d) -> p j d", j=J),
                        axis=AX.X, op=OP.add)

                # ---- softmax over j -> weights C (no max-sub; logits small) ----
                nc.scalar.activation(Ebuf[:], Dbuf[:], ACTF.Exp)
                nc.vector.tensor_reduce(smr[:], Ebuf[:].transpose([0, 2, 1]),
                                        axis=AX.X, op=OP.add)
                rs = st.tile([128, NPAIR], F32, tag="rs")
                nc.vector.reciprocal(rs[:], smr[:])
                nc.vector.tensor_mul(wbuf[:], Ebuf[:],
                                     rs[:].unsqueeze(1).broadcast_to((128, J, NPAIR)))

                # ---- pass B: S = sum_n C*U via XC-folded matmuls per j ----
                for j in range(J):
                    wT = st.tile([128, NPAIR], BF16, tag="wT")
                    nc.sync.dma_start_transpose(wT[:], wbuf[:, j, :])
                    wTd = dram.tile([128, NPAIR], BF16, tag="wTd")
                    nc.sync.dma_start(wTd[:], wT[:])
                    wexp = st.tile([128, NCH, B], BF16, tag="wexp")
                    for nl in range(8):
                        pl, par = nl // 2, nl % 2
                        src = (wTd[pl:128:4, par * B:(par + 1) * B]
                               .unsqueeze(0).broadcast_to((K, NCH, B)))
                        nc.sync.dma_start(wexp[nl * K:(nl + 1) * K, :, :], src)
                    xcw = st.tile([128, NCH, B], BF16, tag="xcw")
                    nc.vector.tensor_mul(xcw[:], XTD[:], wexp[:])
                    sj = psx.tile([D, B], F32, tag="xc")
                    for c in range(NCH):
                        nc.tensor.matmul(sj[:], KF[:, c, j * D:(j + 1) * D],
                                         xcw[:, c, :],
                                         start=(c == 0), stop=(c == NCH - 1))
                    sjs = st.tile([D, B], F32, tag="sjs")
                    nc.vector.tensor_copy(sjs[:], sj[:])
                    stp = psx.tile([B, D], F32, tag="xct")
                    nc.tensor.transpose(stp[:], sjs[:], ID32[:])
                    nc.vector.tensor_copy(Sres[:, j, :], stp[:])

                allreduce(Sres[:].rearrange("b j d -> b (j d)"))
                squash()
                if it == 0:
                    nc.vector.tensor_add(Tt[:], Tt[:], Vt[:])

            nc.sync.dma_start(v_out, Vt[:])

    nc.compile()
    _CACHE["nc"] = nc
    return nc


def _prep_core(x, kern, c):
    """Host-side shard prep for core c. x [B,NI,K] f32, kern [NI,J,K,D] f32."""
    n0 = c * NS
    xs = x[:, n0:n0 + NS, :]                       # [B, NS, K]
    ks = kern[n0:n0 + NS]                          # [NS, J, K, D]
    # KFH [(n k), (j d)] -> [128, NCH*JD] chunk-major partitions
    kf = ks.transpose(0, 2, 1, 3).reshape(NK, JD)  # [(n k), (j d)]
    kf = kf.reshape(NCH, 128, JD).transpose(1, 0, 2).reshape(128, NCH * JD)
    # xTd [(n k), b] -> [128, NCH*B]
    xt = xs.transpose(1, 2, 0).reshape(NK, B)      # [(n k), b]
    xtd = xt.reshape(NCH, 128, B).transpose(1, 0, 2).reshape(128, NCH * B)
    # xT2 block-diag per pair: [32, 128] blocks; tile [128, 32, 128]
    xt2 = np.zeros((128, 32, 128), np.float32)
    for p in range(NPAIR):
        bp = (p % 4) * 32
        g = p // 4
        blk = np.zeros((32, 128), np.float32)
        for nl in range(2):
            n = 2 * p + nl
            # rows nl*16..+16 (k), cols nl*64..+64 (b)
            blk[nl * 16:(nl + 1) * 16, nl * B:(nl + 1) * B] = xs[:, n, :].T
        xt2[bp:bp + 32, g, :] = blk
    bf = ml_dtypes.bfloat16
    return {
        "kfh_in": kf.astype(bf),
        "xt2_in": xt2.reshape(128, 32 * 128).astype(bf),
        "xtd_in": xtd.astype(bf),
        "id32_in": np.eye(32, dtype=np.float32),
    }


def kernel(x, kernel):
    import os
    nc = _build()
    kern = kernel
    in_maps = [_prep_core(np.asarray(x, np.float32),
                          np.asarray(kern, np.float32), c)
               for c in range(N_CORES)]
    trace = bool(int(os.environ.get("KERNEL_TRACE", "0")))
    if trace:
        try:
            res = bass_utils.run_bass_kernel_spmd(
                nc, in_maps, core_ids=list(range(N_CORES)), trace=True)
            if res.exec_time_ns is not None:
                print(f"HW exec time: {res.exec_time_ns} ns")
        except Exception:
            trace = False
    if not trace:
        res = bass_utils.run_bass_kernel_spmd(
            nc, in_maps, core_ids=list(range(N_CORES)))
    out = res.results[0]["v_out"]
    return out.reshape(B, J, D).astype(np.float32)
